# revision 22
# baseline (speedup 1.0000x reference)
"""Trainium2 Bass kernel for nn_Conv2d_14147622273082.

Conv2d 3x3, stride 1, pad 1: x [8, 320, 64, 64] f32, hf8-coded weights
w_bits [320, 320, 3, 3] i32 (codes 0..255), bias codes b_bits [320] i32.
out = conv2d(x, hf8_decode(w_bits)) + hf8_decode(b_bits).

Strategy: data-parallel over batch (1 image per NeuronCore, 8 cores).
Weights replicated; hf8 decode on-device via a bit trick:
hf8(1-4-3, bias 14) == bitcast_f32(sign<<31 | code7<<20) * 2^113
(exact, incl. subnormals). The conv is 9 shifted [Cin,Cout] x [Cin,pix]
fp16 matmuls accumulated in PSUM over a zero-padded fp16 input image.

Cin=320 splits into K-chunks (128, 128, 64). The 64-wide tail would waste
half the PE array, so kernel positions are packed in pairs: partitions
0:64 hold the tail channels, partitions 64:128 hold the same channels
with the padded image pre-shifted by the delta between the two positions
(flat +1 == next column; flat +66 == (row+1, col-2) in the 68-wide pad),
so one K=128 matmul computes two positions at once. 9 positions -> 4
pairs + 1 solo: 23 instead of 27 accumulating matmuls per PSUM tile.
"""

import numpy as np

import concourse.bass as bass
import concourse.tile as tile
from concourse import bacc, mybir
from concourse.bass_utils import run_bass_kernel_spmd

B, CIN, COUT, H, W = 8, 320, 320, 64, 64
PIX = H * W  # 4096
P = 128
CO_CHUNKS = [(0, 128), (128, 256), (256, 320)]
N_TILE = 512  # pixels per psum tile = 8 rows of 64
ROWS_PER_TILE = N_TILE // W  # 8
N_PIX_TILES = PIX // N_TILE  # 8
# padded image: rows 0..65 (top/bottom zero), cols: 2 left / 2 right zero
HP, WP = H + 2, W + 4  # 66 x 68 (even left pad keeps fp16 writes 4B-aligned)

# tail position pairing: (pos_a, pos_b) packed on partitions (0:64, 64:128).
# delta = flat_offset(b) - flat_offset(a) in the padded [66,68] layout.
# pairs with delta 1 share the "+1 shifted" upper image (xp2 upper half);
# the delta-66 pair gets its own tile (XB).
TAIL_PAIRS = [(0, 1), (2, 3), (4, 5), (6, 7)]  # pos = kh*3+kw
TAIL_SOLO = 8

F16 = mybir.dt.float16
F32 = mybir.dt.float32
I32 = mybir.dt.int32
HF8_SCALE = float(2.0**113)


def _decode_hf8(nc, pool, codes_ap, out_ap, nparts, free, tag):
    """out = hf8_decode(codes) = bitcast_f32(sign<<31 | code7<<20) * 2^113."""
    t1 = pool.tile([P, free], I32, tag=f"{tag}_t1", name=f"{tag}_t1")
    t2 = pool.tile([P, free], I32, tag=f"{tag}_t2", name=f"{tag}_t2")
    nc.vector.tensor_scalar(
        t1[:nparts], codes_ap, 0x80, 24,
        mybir.AluOpType.bitwise_and, mybir.AluOpType.logical_shift_left,
    )
    nc.vector.tensor_scalar(
        t2[:nparts], codes_ap, 0x7F, 20,
        mybir.AluOpType.bitwise_and, mybir.AluOpType.logical_shift_left,
    )
    nc.vector.tensor_tensor(
        t1[:nparts], t1[:nparts], t2[:nparts], mybir.AluOpType.bitwise_or
    )
    nc.vector.tensor_scalar_mul(out_ap, t1[:nparts].bitcast(F32), HF8_SCALE)


def _pad_borders(nc, xt, col_lo, col_hi, parts=slice(0, P), rows=(0, HP - 1)):
    """Zero the pad borders around an interior written at cols [col_lo, col_hi)."""
    nc.vector.memset(xt[parts, rows[0] : rows[0] + 1, :], 0.0)
    nc.vector.memset(xt[parts, rows[1] : rows[1] + 1, :], 0.0)
    if col_lo > 0:
        nc.vector.memset(xt[parts, rows[0] + 1 : rows[1], 0:col_lo], 0.0)
    if col_hi < WP:
        nc.vector.memset(xt[parts, rows[0] + 1 : rows[1], col_hi:WP], 0.0)


def build():
    nc = bacc.Bacc(
        "TRN2", target_bir_lowering=False, debug=False, enable_partition_id=False
    )
    x_d = nc.dram_tensor("x", [CIN, PIX], F32, kind="ExternalInput")
    w_d = nc.dram_tensor("w9", [CIN, 9, COUT], I32, kind="ExternalInput")
    b_d = nc.dram_tensor("b", [COUT, 1], I32, kind="ExternalInput")
    out_d = nc.dram_tensor("out", [COUT, PIX], F32, kind="ExternalOutput")

    with tile.TileContext(nc) as tc:
        with (
            tc.tile_pool(name="persist", bufs=1) as persist,
            tc.tile_pool(name="stage", bufs=1) as stage,
            tc.tile_pool(name="outsb", bufs=4) as outsb,
            tc.tile_pool(name="psum", bufs=1, space="PSUM") as psum_pool,
        ):
            # Engine/queue split: weight+bias DMAs on the sync queue, image
            # DMAs AND pad casts on gpsimd (in-order, so later DMAs/casts
            # self-throttle behind earlier ones), hf8 decode on DVE, PSUM
            # epilogue on the Scalar engine. stage pool bufs=1 makes each
            # later input DMA wait until the previous chunk is consumed,
            # keeping early HBM bandwidth for the critical chunk-0 path.
            wl = [None, None]
            xp = [None, None]
            bias = []

            # warm up the Scalar engine's Copy table while DMAs are in flight
            warm = stage.tile([P, 1], F32, tag="warm", name="warm")
            nc.vector.memset(warm[:], 0.0)
            nc.scalar.copy(warm[:], warm[:])

            # PE warmup: dummy matmuls keep TensorE busy through the DMA/
            # decode prologue so the HAM clock gate is at 8/8 (2.4 GHz) when
            # the real stream starts, instead of ramping for ~3.4us. They
            # write a scratch view sharing the acc0 PSUM slot (released
            # before the first real accumulation group begins).
            wsrc = stage.tile([P, P], F16, tag="wsrc", name="wsrc")
            nc.vector.memset(wsrc[:], 0.0)
            warm_ps = psum_pool.tile([P, N_TILE], F32, tag="acc0", name="warm_ps")
            for _ in range(180):
                nc.tensor.matmul(
                    warm_ps[:, 0:P], wsrc[:], wsrc[:], start=True, stop=True
                )

            # ---- chunk 0 / 1 (full 128-channel ci chunks) ----
            for ci in range(2):
                cs, ce = ci * P, (ci + 1) * P
                wraw = stage.tile([P, 9, COUT], I32, tag=f"wraw{ci}", name="wraw")
                wt = persist.tile([P, 9, COUT], F16, tag=f"wl{ci}", name=f"wl{ci}")
                xs = stage.tile([P, H, W], F32, tag=f"xstage{ci}", name="xstage")
                xt = persist.tile([P, HP, WP], F16, tag=f"xpad{ci}", name=f"xpad{ci}")
                _pad_borders(nc, xt, 2, W + 2)
                wflat = wraw.rearrange("p a b -> p (a b)")
                oflat = wt.rearrange("p a b -> p (a b)")
                if ci == 0:
                    # halves: decode/cast start as soon as the first half lands
                    half = 5 * COUT
                    nc.sync.dma_start(wflat[:, :half], w_d[cs:ce, :5])
                    nc.gpsimd.dma_start(
                        xs[:, : H // 2],
                        x_d[cs:ce, : PIX // 2].rearrange("p (h w) -> p h w", h=H // 2),
                    )
                    nc.sync.dma_start(wflat[:, half:], w_d[cs:ce, 5:])
                    nc.gpsimd.dma_start(
                        xs[:, H // 2 :],
                        x_d[cs:ce, PIX // 2 :].rearrange("p (h w) -> p h w", h=H // 2),
                    )
                    nc.scalar.copy(
                        xt[:, 1 : H // 2 + 1, 2 : W + 2], xs[:, : H // 2]
                    )
                    nc.scalar.copy(
                        xt[:, H // 2 + 1 : H + 1, 2 : W + 2], xs[:, H // 2 :]
                    )
                    _decode_hf8(nc, stage, wflat[:, :half], oflat[:, :half],
                                P, half, "wdec")
                    _decode_hf8(nc, stage, wflat[:, half:], oflat[:, half:],
                                P, 9 * COUT - half, "wdec")
                else:
                    nc.sync.dma_start(wraw[:], w_d[cs:ce])
                    nc.gpsimd.dma_start(
                        xs[:], x_d[cs:ce].rearrange("p (h w) -> p h w", h=H)
                    )
                    nc.scalar.copy(xt[:, 1 : H + 1, 2 : W + 2], xs[:])
                    _decode_hf8(nc, stage, wflat, oflat, P, 9 * COUT, "wdec")
                wl[ci] = wt
                xp[ci] = xt

            # ---- tail chunk (ci 256:320, 64 channels) with position pairing ----
            cs, ce = 256, 320
            wraw2 = stage.tile([P, 5, COUT], I32, tag="wraw2", name="wraw2")
            for j, (pa, pb) in enumerate(TAIL_PAIRS):
                nc.sync.dma_start(wraw2[0:64, j], w_d[cs:ce, pa])
                nc.sync.dma_start(wraw2[64:128, j], w_d[cs:ce, pb])
            nc.sync.dma_start(wraw2[0:64, 4], w_d[cs:ce, TAIL_SOLO])
            nc.vector.memset(wraw2[64:128, 4], 0)
            wpair = persist.tile([P, 5, COUT], F16, tag="wpair", name="wpair")
            _decode_hf8(
                nc, stage,
                wraw2.rearrange("p a b -> p (a b)"),
                wpair.rearrange("p a b -> p (a b)"),
                P, 5 * COUT, "wdec2",
            )

            # tail image staged twice (lower + upper partition halves)
            xs2 = stage.tile([P, H, W], F32, tag="xstage", name="xstage2")
            nc.gpsimd.dma_start(
                xs2[0:64], x_d[cs:ce].rearrange("p (h w) -> p h w", h=H)
            )
            nc.gpsimd.dma_start(
                xs2[64:128], x_d[cs:ce].rearrange("p (h w) -> p h w", h=H)
            )

            # Reading the upper half with pos_a's window offsets must yield
            # pos_b's window: place the upper interior at (1-dkh, 2-dkw).
            # xp2: lower = padded image (ACT cast); upper = interior at
            # (1, 1) for the (dkh,dkw)=(0,1) pairs. xb2: lower = padded
            # image; upper = interior at (0, 4) for the (2,3) pair. The
            # three derived interiors are partition-shifted copies of the
            # casted lower image, done with SBUF->SBUF DMAs.
            xp2 = persist.tile([P, HP, WP], F16, tag="xpad2", name="xpad2")
            _pad_borders(nc, xp2, 2, W + 2, parts=slice(0, 64))
            _pad_borders(nc, xp2, 1, W + 1, parts=slice(64, P))
            nc.scalar.copy(xp2[0:64, 1 : H + 1, 2 : W + 2], xs2[0:64])
            nc.scalar.copy(xp2[64:128, 1 : H + 1, 1 : W + 1], xs2[64:128])

            xb2 = persist.tile([P, HP, WP], F16, tag="xpadb", name="xpadb")
            _pad_borders(nc, xb2, 2, W + 2, parts=slice(0, 64))
            nc.vector.memset(xb2[64:128, H : HP, :], 0.0)
            nc.vector.memset(xb2[64:128, 0:H, 0:4], 0.0)
            nc.vector.tensor_copy(
                xb2[0:64, 1 : H + 1, 2 : W + 2], xp2[0:64, 1 : H + 1, 2 : W + 2]
            )
            nc.scalar.copy(xb2[64:128, 0:H, 4:WP], xs2[64:128])

            # ---- bias: [320,1] i32 -> three [p,1] f32 tiles ----
            for mi, (ms, me) in enumerate(CO_CHUNKS):
                pm = me - ms
                braw = stage.tile([P, 1], I32, tag="braw", name="braw")
                nc.sync.dma_start(braw[:pm], b_d[ms:me, :])
                bf = persist.tile([P, 1], F32, tag=f"bias{mi}", name=f"bias{mi}")
                _decode_hf8(nc, stage, braw[:pm], bf[:pm], pm, 1, "bdec")
                bias.append(bf)

            # ---- matmuls: out[co, pix] += w[ci,co].T @ x_shift[ci, pix] ----
            n_acc = 2 * 9 + len(TAIL_PAIRS) + 1  # 23 per psum tile
            for mi, (ms, me) in enumerate(CO_CHUNKS):
                pm = me - ms
                acc = [
                    psum_pool.tile(
                        [P, N_TILE], F32, tag=f"acc{t}", name=f"acc_{mi}_{t}"
                    )
                    for t in range(N_PIX_TILES)
                ]
                acc_k = [0] * N_PIX_TILES

                def mm(lhsT, src, kh, kw, t, pm=pm, acc=acc, acc_k=acc_k):
                    h0 = t * ROWS_PER_TILE
                    rhs = src[
                        : lhsT.shape[0],
                        h0 + kh : h0 + kh + ROWS_PER_TILE,
                        kw + 1 : kw + 1 + W,
                    ]
                    nc.tensor.matmul(
                        acc[t][:pm], lhsT, rhs,
                        start=(acc_k[t] == 0), stop=(acc_k[t] == n_acc - 1),
                    )
                    acc_k[t] += 1

                # For the very first co chunk, order chunk-0 work as
                # (weight half x image half) passes: the first 20 matmuls
                # need only the first 5 decoded positions and the first half
                # of the chunk-0 image, so the stream starts as soon as
                # x0h1 + the first weight half land.
                if mi == 0:
                    c0_passes = [
                        (range(0, 5), range(0, 4)),
                        (range(5, 9), range(0, 4)),
                        (range(0, 5), range(4, 8)),
                        (range(5, 9), range(4, 8)),
                    ]
                else:
                    c0_passes = [(range(9), range(N_PIX_TILES))]
                for ci in range(2):
                    passes = c0_passes if ci == 0 else [(range(9), range(N_PIX_TILES))]
                    for pos_range, t_range in passes:
                        for pos in pos_range:
                            lhsT = wl[ci][:, pos, ms:me]
                            for t in t_range:
                                mm(lhsT, xp[ci], pos // 3, pos % 3, t)
                # paired tail positions: K=128, upper half pre-shifted
                for j, (pa, pb) in enumerate(TAIL_PAIRS):
                    kh, kw = pa // 3, pa % 3
                    src = xb2 if (pa, pb) == (2, 3) else xp2
                    lhsT = wpair[:, j, ms:me]
                    for t in range(N_PIX_TILES):
                        mm(lhsT, src, kh, kw, t)
                # solo tail position (2,2): K=64
                lhsT = wpair[0:64, 4, ms:me]
                for t in range(N_PIX_TILES):
                    mm(lhsT, xp2, 2, 2, t)
                assert all(k == n_acc for k in acc_k)

                for t in range(N_PIX_TILES):
                    osb = outsb.tile([P, N_TILE], F32, tag="osb", name="osb")
                    nc.scalar.activation(
                        osb[:pm], acc[t][:pm],
                        mybir.ActivationFunctionType.Identity,
                        bias=bias[mi][:pm], scale=1.0,
                    )
                    nc.sync.dma_start(
                        out_d[ms:me, t * N_TILE : (t + 1) * N_TILE], osb[:pm]
                    )

    nc.compile()
    return nc


_NC_CACHE = None


def _get_nc():
    global _NC_CACHE
    if _NC_CACHE is None:
        _NC_CACHE = build()
    return _NC_CACHE


def _prep_in_maps(x, w_bits, b_bits):
    # w_bits [co, ci, kh, kw] -> [ci, kh*3+kw, co] (host relayout only)
    w9 = np.ascontiguousarray(
        w_bits.astype(np.int32).transpose(1, 2, 3, 0).reshape(CIN, 9, COUT)
    )
    b2 = np.ascontiguousarray(b_bits.astype(np.int32).reshape(COUT, 1))
    return [
        {
            "x": np.ascontiguousarray(x[i].reshape(CIN, PIX).astype(np.float32)),
            "w9": w9,
            "b": b2,
        }
        for i in range(B)
    ]


def kernel(x, w_bits, b_bits):
    nc = _get_nc()
    in_maps = _prep_in_maps(x, w_bits, b_bits)
    res = run_bass_kernel_spmd(nc, in_maps, core_ids=list(range(B)), trace=False)
    return np.stack(
        [res.results[i]["out"].reshape(COUT, H, W) for i in range(B)]
    ).astype(np.float32)


if __name__ == "__main__":
    rng = np.random.default_rng(0)
    x = rng.standard_normal((B, CIN, H, W)).astype(np.float32)
    w_bits = rng.integers(0, 256, (COUT, CIN, 3, 3)).astype(np.int32)
    b_bits = rng.integers(0, 256, (COUT,)).astype(np.int32)
    out = kernel(x, w_bits, b_bits)
    print("out", out.shape, out.dtype, float(np.abs(out).mean()))


# revision 23
# speedup vs baseline: 1.1244x; 1.1244x over previous
"""Trainium2 Bass kernel for nn_Conv2d_14147622273082.

Conv2d 3x3, stride 1, pad 1: x [8, 320, 64, 64] f32, hf8-coded weights
w_bits [320, 320, 3, 3] i32 (codes 0..255), bias codes b_bits [320] i32.
out = conv2d(x, hf8_decode(w_bits)) + hf8_decode(b_bits).

Strategy: data-parallel over batch (1 image per NeuronCore, 8 cores).
Weights replicated; hf8 decode on-device via a bit trick:
hf8(1-4-3, bias 14) == bitcast_f32(sign<<31 | code7<<20) * 2^113
(exact, incl. subnormals). The conv is 9 shifted [Cin,Cout] x [Cin,pix]
fp16 matmuls accumulated in PSUM over a zero-padded fp16 input image.

Cin=320 splits into K-chunks (128, 128, 64). The 64-wide tail would waste
half the PE array, so kernel positions are packed in pairs: partitions
0:64 hold the tail channels, partitions 64:128 hold the same channels
with the padded image pre-shifted by the delta between the two positions
(flat +1 == next column; flat +66 == (row+1, col-2) in the 68-wide pad),
so one K=128 matmul computes two positions at once. 9 positions -> 4
pairs + 1 solo: 23 instead of 27 accumulating matmuls per PSUM tile.
"""

import numpy as np

import concourse.bass as bass
import concourse.tile as tile
from concourse import bacc, mybir
from concourse.bass_utils import run_bass_kernel_spmd

B, CIN, COUT, H, W = 8, 320, 320, 64, 64
PIX = H * W  # 4096
P = 128
CO_CHUNKS = [(0, 128), (128, 256), (256, 320)]
N_TILE = 512  # pixels per psum tile = 8 rows of 64
ROWS_PER_TILE = N_TILE // W  # 8
N_PIX_TILES = PIX // N_TILE  # 8
# padded image: rows 0..65 (top/bottom zero), cols: 2 left / 2 right zero
HP, WP = H + 2, W + 4  # 66 x 68 (even left pad keeps fp16 writes 4B-aligned)

# tail position pairing: (pos_a, pos_b) packed on partitions (0:64, 64:128).
# delta = flat_offset(b) - flat_offset(a) in the padded [66,68] layout.
# pairs with delta 1 share the "+1 shifted" upper image (xp2 upper half);
# the delta-66 pair gets its own tile (XB).
TAIL_PAIRS = [(0, 1), (2, 3), (4, 5), (6, 7)]  # pos = kh*3+kw
TAIL_SOLO = 8

F16 = mybir.dt.float16
F32 = mybir.dt.float32
I32 = mybir.dt.int32
HF8_SCALE = float(2.0**113)


def _decode_hf8(nc, pool, codes_ap, out_ap, nparts, free, tag, after=None):
    """out = hf8_decode(codes) = bitcast_f32(sign<<31 | code7<<20) * 2^113.

    Returns the last instruction. If `after` is given, the stage's first ops
    get no-sync ordering edges onto it so the Tile scheduler cannot hoist
    this stage ahead of earlier work on the engine (its compile-time DMA
    timing model underestimates HBM contention, which otherwise causes
    head-of-line stalls).
    """
    from concourse.tile_rust import add_dep_helper

    t1 = pool.tile([P, free], I32, tag=f"{tag}_t1", name=f"{tag}_t1")
    t2 = pool.tile([P, free], I32, tag=f"{tag}_t2", name=f"{tag}_t2")
    i1 = nc.vector.tensor_scalar(
        t1[:nparts], codes_ap, 0x80, 24,
        mybir.AluOpType.bitwise_and, mybir.AluOpType.logical_shift_left,
    )
    i2 = nc.vector.tensor_scalar(
        t2[:nparts], codes_ap, 0x7F, 20,
        mybir.AluOpType.bitwise_and, mybir.AluOpType.logical_shift_left,
    )
    if after is not None:
        add_dep_helper(i1.ins, after.ins, sync=False, reason="decode stage order")
        add_dep_helper(i2.ins, after.ins, sync=False, reason="decode stage order")
    nc.vector.tensor_tensor(
        t1[:nparts], t1[:nparts], t2[:nparts], mybir.AluOpType.bitwise_or
    )
    return nc.vector.tensor_scalar_mul(out_ap, t1[:nparts].bitcast(F32), HF8_SCALE)


def _pad_borders(nc, xt, col_lo, col_hi, parts=slice(0, P), rows=(0, HP - 1)):
    """Zero the pad borders around an interior written at cols [col_lo, col_hi)."""
    nc.vector.memset(xt[parts, rows[0] : rows[0] + 1, :], 0.0)
    nc.vector.memset(xt[parts, rows[1] : rows[1] + 1, :], 0.0)
    if col_lo > 0:
        nc.vector.memset(xt[parts, rows[0] + 1 : rows[1], 0:col_lo], 0.0)
    if col_hi < WP:
        nc.vector.memset(xt[parts, rows[0] + 1 : rows[1], col_hi:WP], 0.0)


def build():
    from concourse.tile_rust import add_dep_helper

    nc = bacc.Bacc(
        "TRN2", target_bir_lowering=False, debug=False, enable_partition_id=False
    )
    x_d = nc.dram_tensor("x", [CIN, PIX], F32, kind="ExternalInput")
    w_d = nc.dram_tensor("w9", [CIN, 9, COUT], I32, kind="ExternalInput")
    b_d = nc.dram_tensor("b", [COUT, 1], I32, kind="ExternalInput")
    out_d = nc.dram_tensor("out", [COUT, PIX], F32, kind="ExternalOutput")

    with tile.TileContext(nc) as tc:
        with (
            tc.tile_pool(name="persist", bufs=1) as persist,
            tc.tile_pool(name="stage", bufs=1) as stage,
            tc.tile_pool(name="outsb", bufs=4) as outsb,
            tc.tile_pool(name="psum", bufs=1, space="PSUM") as psum_pool,
        ):
            # All input DMAs ride the sync queue, which processes them in
            # issue order: earliest-deadline first. hf8 decode runs on DVE,
            # pad casts on the Scalar engine, PSUM epilogue on Scalar.
            # ---- SBUF tiles ----
            wraw = [
                stage.tile([P, 9, COUT], I32, tag=f"wraw{c}", name=f"wraw{c}")
                for c in range(2)
            ]
            wt = [
                persist.tile([P, 9, COUT], F16, tag=f"wl{c}", name=f"wl{c}")
                for c in range(2)
            ]
            xs = [
                stage.tile([P, H, W], F32, tag=f"xstage{c}", name=f"xstage{c}")
                for c in range(2)
            ]
            xt = [
                persist.tile([P, HP, WP], F16, tag=f"xpad{c}", name=f"xpad{c}")
                for c in range(2)
            ]
            wraw2 = stage.tile([P, 5, COUT], I32, tag="wraw2", name="wraw2")
            wpair = persist.tile([P, 5, COUT], F16, tag="wpair", name="wpair")
            xs2 = stage.tile([P, H, W], F32, tag="xstage2", name="xstage2")
            xp2 = persist.tile([P, HP, WP], F16, tag="xpad2", name="xpad2")
            xb2 = persist.tile([P, HP, WP], F16, tag="xpadb", name="xpadb")
            wl = wt
            xp = xt

            # ---- input DMAs, deadline order, one in-order queue ----
            wfl = [w.rearrange("p a b -> p (a b)") for w in wraw]
            half = 5 * COUT
            nc.sync.dma_start(wfl[0][:, :half], w_d[0:P, :5])
            nc.sync.dma_start(
                xs[0][:, : H // 2],
                x_d[0:P, : PIX // 2].rearrange("p (h w) -> p h w", h=H // 2),
            )
            nc.sync.dma_start(wfl[0][:, half:], w_d[0:P, 5:])
            nc.sync.dma_start(
                xs[0][:, H // 2 :],
                x_d[0:P, PIX // 2 :].rearrange("p (h w) -> p h w", h=H // 2),
            )
            nc.sync.dma_start(wraw[1][:], w_d[P : 2 * P])
            nc.sync.dma_start(
                xs[1][:], x_d[P : 2 * P].rearrange("p (h w) -> p h w", h=H)
            )
            cs, ce = 256, 320
            for j, (pa, pb) in enumerate(TAIL_PAIRS):
                nc.sync.dma_start(wraw2[0:64, j], w_d[cs:ce, pa])
                nc.sync.dma_start(wraw2[64:128, j], w_d[cs:ce, pb])
            nc.sync.dma_start(wraw2[0:64, 4], w_d[cs:ce, TAIL_SOLO])
            nc.sync.dma_start(
                xs2[0:64], x_d[cs:ce].rearrange("p (h w) -> p h w", h=H)
            )
            nc.sync.dma_start(
                xs2[64:128], x_d[cs:ce].rearrange("p (h w) -> p h w", h=H)
            )
            braw = stage.tile([P, 3], I32, tag="braw", name="braw")
            for mi, (ms, me) in enumerate(CO_CHUNKS):
                nc.sync.dma_start(braw[: me - ms, mi : mi + 1], b_d[ms:me, :])

            # ---- PE warmup: keep TensorE busy (HAM at 8/8) through the
            # prologue so the real stream starts at 2.4 GHz ----
            wsrc = stage.tile([P, P], F16, tag="wsrc", name="wsrc")
            nc.vector.memset(wsrc[:], 0.0)
            warm_ps = psum_pool.tile([P, N_TILE], F32, tag="acc0", name="warm_ps")
            for _ in range(180):
                nc.tensor.matmul(
                    warm_ps[:, 0:P], wsrc[:], wsrc[:], start=True, stop=True
                )

            # ---- borders (DVE, no data deps: fills the DMA wait) ----
            for c in range(2):
                _pad_borders(nc, xt[c], 2, W + 2)
            _pad_borders(nc, xp2, 2, W + 2, parts=slice(0, 64))
            _pad_borders(nc, xp2, 1, W + 1, parts=slice(64, P))
            _pad_borders(nc, xb2, 2, W + 2, parts=slice(0, 64))
            nc.vector.memset(xb2[64:128, H : HP, :], 0.0)
            nc.vector.memset(xb2[64:128, 0:H, 0:4], 0.0)
            nc.vector.memset(wraw2[64:128, 4], 0)

            # ---- Scalar-engine casts (warm the Copy table first), chained
            # in deadline order so the static schedule matches reality ----
            warm = stage.tile([P, 1], F32, tag="warm", name="warm")
            nc.vector.memset(warm[:], 0.0)
            a0 = nc.scalar.copy(warm[:], warm[:])
            a1 = nc.scalar.copy(
                xt[0][:, 1 : H // 2 + 1, 2 : W + 2], xs[0][:, : H // 2]
            )
            a2 = nc.scalar.copy(
                xt[0][:, H // 2 + 1 : H + 1, 2 : W + 2], xs[0][:, H // 2 :]
            )
            a3 = nc.scalar.copy(xt[1][:, 1 : H + 1, 2 : W + 2], xs[1][:])
            a4 = nc.scalar.copy(xp2[0:64, 1 : H + 1, 2 : W + 2], xs2[0:64])
            a5 = nc.scalar.copy(xp2[64:128, 1 : H + 1, 1 : W + 1], xs2[64:128])
            a6 = nc.scalar.copy(xb2[64:128, 0:H, 4:WP], xs2[64:128])
            prev = a0
            for a in (a1, a2, a3, a4, a5, a6):
                add_dep_helper(a.ins, prev.ins, sync=False, reason="cast order")
                prev = a

            # ---- hf8 decode on DVE, stage-chained in deadline order ----
            d1 = _decode_hf8(
                nc, stage, wfl[0][:, :half],
                wt[0].rearrange("p a b -> p (a b)")[:, :half], P, half, "wdec",
            )
            d2 = _decode_hf8(
                nc, stage, wfl[0][:, half:],
                wt[0].rearrange("p a b -> p (a b)")[:, half:],
                P, 9 * COUT - half, "wdec", after=d1,
            )
            d3 = _decode_hf8(
                nc, stage, wfl[1],
                wt[1].rearrange("p a b -> p (a b)"), P, 9 * COUT, "wdec", after=d2,
            )
            d4 = _decode_hf8(
                nc, stage,
                wraw2.rearrange("p a b -> p (a b)"),
                wpair.rearrange("p a b -> p (a b)"), P, 5 * COUT, "wdec2", after=d3,
            )
            # xb2 lower = same padded image as xp2 lower (same partitions)
            cpy = nc.vector.tensor_copy(
                xb2[0:64, 1 : H + 1, 2 : W + 2], xp2[0:64, 1 : H + 1, 2 : W + 2]
            )
            add_dep_helper(cpy.ins, d4.ins, sync=False, reason="tail copy order")
            bias = []
            prev = None
            for mi, (ms, me) in enumerate(CO_CHUNKS):
                pm = me - ms
                bf = persist.tile([P, 1], F32, tag=f"bias{mi}", name=f"bias{mi}")
                prev = _decode_hf8(
                    nc, stage, braw[:pm, mi : mi + 1], bf[:pm], pm, 1, "bdec",
                    after=prev if prev is not None else d4,
                )
                bias.append(bf)

            # ---- matmuls: out[co, pix] += w[ci,co].T @ x_shift[ci, pix] ----
            n_acc = 2 * 9 + len(TAIL_PAIRS) + 1  # 23 per psum tile
            for mi, (ms, me) in enumerate(CO_CHUNKS):
                pm = me - ms
                acc = [
                    psum_pool.tile(
                        [P, N_TILE], F32, tag=f"acc{t}", name=f"acc_{mi}_{t}"
                    )
                    for t in range(N_PIX_TILES)
                ]
                acc_k = [0] * N_PIX_TILES

                def mm(lhsT, src, kh, kw, t, pm=pm, acc=acc, acc_k=acc_k):
                    h0 = t * ROWS_PER_TILE
                    rhs = src[
                        : lhsT.shape[0],
                        h0 + kh : h0 + kh + ROWS_PER_TILE,
                        kw + 1 : kw + 1 + W,
                    ]
                    nc.tensor.matmul(
                        acc[t][:pm], lhsT, rhs,
                        start=(acc_k[t] == 0), stop=(acc_k[t] == n_acc - 1),
                    )
                    acc_k[t] += 1

                # For the very first co chunk, order chunk-0 work as
                # (weight half x image half) passes: the first 20 matmuls
                # need only the first 5 decoded positions and the first half
                # of the chunk-0 image.
                if mi == 0:
                    c0_passes = [
                        (range(0, 5), range(0, 4)),
                        (range(5, 9), range(0, 4)),
                        (range(0, 5), range(4, 8)),
                        (range(5, 9), range(4, 8)),
                    ]
                else:
                    c0_passes = [(range(9), range(N_PIX_TILES))]
                for ci in range(2):
                    passes = c0_passes if ci == 0 else [(range(9), range(N_PIX_TILES))]
                    for pos_range, t_range in passes:
                        for pos in pos_range:
                            lhsT = wl[ci][:, pos, ms:me]
                            for t in t_range:
                                mm(lhsT, xp[ci], pos // 3, pos % 3, t)
                # paired tail positions: K=128, upper half pre-shifted
                for j, (pa, pb) in enumerate(TAIL_PAIRS):
                    kh, kw = pa // 3, pa % 3
                    src = xb2 if (pa, pb) == (2, 3) else xp2
                    lhsT = wpair[:, j, ms:me]
                    for t in range(N_PIX_TILES):
                        mm(lhsT, src, kh, kw, t)
                # solo tail position (2,2): K=64
                lhsT = wpair[0:64, 4, ms:me]
                for t in range(N_PIX_TILES):
                    mm(lhsT, xp2, 2, 2, t)
                assert all(k == n_acc for k in acc_k)

                for t in range(N_PIX_TILES):
                    osb = outsb.tile([P, N_TILE], F32, tag="osb", name="osb")
                    nc.scalar.activation(
                        osb[:pm], acc[t][:pm],
                        mybir.ActivationFunctionType.Identity,
                        bias=bias[mi][:pm], scale=1.0,
                    )
                    nc.sync.dma_start(
                        out_d[ms:me, t * N_TILE : (t + 1) * N_TILE], osb[:pm]
                    )

    nc.compile()
    return nc


_NC_CACHE = None


def _get_nc():
    global _NC_CACHE
    if _NC_CACHE is None:
        _NC_CACHE = build()
    return _NC_CACHE


def _prep_in_maps(x, w_bits, b_bits):
    # w_bits [co, ci, kh, kw] -> [ci, kh*3+kw, co] (host relayout only)
    w9 = np.ascontiguousarray(
        w_bits.astype(np.int32).transpose(1, 2, 3, 0).reshape(CIN, 9, COUT)
    )
    b2 = np.ascontiguousarray(b_bits.astype(np.int32).reshape(COUT, 1))
    return [
        {
            "x": np.ascontiguousarray(x[i].reshape(CIN, PIX).astype(np.float32)),
            "w9": w9,
            "b": b2,
        }
        for i in range(B)
    ]


def kernel(x, w_bits, b_bits):
    nc = _get_nc()
    in_maps = _prep_in_maps(x, w_bits, b_bits)
    res = run_bass_kernel_spmd(nc, in_maps, core_ids=list(range(B)), trace=False)
    return np.stack(
        [res.results[i]["out"].reshape(COUT, H, W) for i in range(B)]
    ).astype(np.float32)


if __name__ == "__main__":
    rng = np.random.default_rng(0)
    x = rng.standard_normal((B, CIN, H, W)).astype(np.float32)
    w_bits = rng.integers(0, 256, (COUT, CIN, 3, 3)).astype(np.int32)
    b_bits = rng.integers(0, 256, (COUT,)).astype(np.int32)
    out = kernel(x, w_bits, b_bits)
    print("out", out.shape, out.dtype, float(np.abs(out).mean()))


# revision 25
# speedup vs baseline: 1.1332x; 1.0078x over previous
"""Trainium2 Bass kernel for nn_Conv2d_14147622273082.

Conv2d 3x3, stride 1, pad 1: x [8, 320, 64, 64] f32, hf8-coded weights
w_bits [320, 320, 3, 3] i32 (codes 0..255), bias codes b_bits [320] i32.
out = conv2d(x, hf8_decode(w_bits)) + hf8_decode(b_bits).

Strategy: data-parallel over batch (1 image per NeuronCore, 8 cores).
Weights replicated; hf8 decode on-device via a bit trick:
hf8(1-4-3, bias 14) == bitcast_f32(sign<<31 | code7<<20) * 2^113
(exact, incl. subnormals). The conv is 9 shifted [Cin,Cout] x [Cin,pix]
fp16 matmuls accumulated in PSUM over a zero-padded fp16 input image.

Cin=320 splits into K-chunks (128, 128, 64). The 64-wide tail would waste
half the PE array, so kernel positions are packed in pairs: partitions
0:64 hold the tail channels, partitions 64:128 hold the same channels
with the padded image pre-shifted by the delta between the two positions
(flat +1 == next column; flat +66 == (row+1, col-2) in the 68-wide pad),
so one K=128 matmul computes two positions at once. 9 positions -> 4
pairs + 1 solo: 23 instead of 27 accumulating matmuls per PSUM tile.
"""

import numpy as np

import concourse.bass as bass
import concourse.tile as tile
from concourse import bacc, mybir
from concourse.bass_utils import run_bass_kernel_spmd

B, CIN, COUT, H, W = 8, 320, 320, 64, 64
PIX = H * W  # 4096
P = 128
CO_CHUNKS = [(0, 128), (128, 256), (256, 320)]
N_TILE = 512  # pixels per psum tile = 8 rows of 64
ROWS_PER_TILE = N_TILE // W  # 8
N_PIX_TILES = PIX // N_TILE  # 8
# padded image: rows 0..65 (top/bottom zero), cols: 2 left / 2 right zero
HP, WP = H + 2, W + 4  # 66 x 68 (even left pad keeps fp16 writes 4B-aligned)

# tail position pairing: (pos_a, pos_b) packed on partitions (0:64, 64:128).
# delta = flat_offset(b) - flat_offset(a) in the padded [66,68] layout.
# pairs with delta 1 share the "+1 shifted" upper image (xp2 upper half);
# the delta-66 pair gets its own tile (XB).
TAIL_PAIRS = [(0, 1), (2, 3), (4, 5), (6, 7)]  # pos = kh*3+kw
TAIL_SOLO = 8

F16 = mybir.dt.float16
F32 = mybir.dt.float32
I32 = mybir.dt.int32
HF8_SCALE = float(2.0**113)


def _decode_hf8(nc, pool, codes_ap, out_ap, nparts, free, tag, after=None):
    """out = hf8_decode(codes) = bitcast_f32(sign<<31 | code7<<20) * 2^113.

    Returns the last instruction. If `after` is given, the stage's first ops
    get no-sync ordering edges onto it so the Tile scheduler cannot hoist
    this stage ahead of earlier work on the engine (its compile-time DMA
    timing model underestimates HBM contention, which otherwise causes
    head-of-line stalls).
    """
    from concourse.tile_rust import add_dep_helper

    t1 = pool.tile([P, free], I32, tag=f"{tag}_t1", name=f"{tag}_t1")
    t2 = pool.tile([P, free], I32, tag=f"{tag}_t2", name=f"{tag}_t2")
    i1 = nc.vector.tensor_scalar(
        t1[:nparts], codes_ap, 0x80, 24,
        mybir.AluOpType.bitwise_and, mybir.AluOpType.logical_shift_left,
    )
    i2 = nc.vector.tensor_scalar(
        t2[:nparts], codes_ap, 0x7F, 20,
        mybir.AluOpType.bitwise_and, mybir.AluOpType.logical_shift_left,
    )
    if after is not None:
        add_dep_helper(i1.ins, after.ins, sync=False, reason="decode stage order")
        add_dep_helper(i2.ins, after.ins, sync=False, reason="decode stage order")
    nc.vector.tensor_tensor(
        t1[:nparts], t1[:nparts], t2[:nparts], mybir.AluOpType.bitwise_or
    )
    return nc.vector.tensor_scalar_mul(out_ap, t1[:nparts].bitcast(F32), HF8_SCALE)


def _pad_borders(nc, xt, col_lo, col_hi, parts=slice(0, P), rows=(0, HP - 1)):
    """Zero the pad borders around an interior written at cols [col_lo, col_hi)."""
    nc.vector.memset(xt[parts, rows[0] : rows[0] + 1, :], 0.0)
    nc.vector.memset(xt[parts, rows[1] : rows[1] + 1, :], 0.0)
    if col_lo > 0:
        nc.vector.memset(xt[parts, rows[0] + 1 : rows[1], 0:col_lo], 0.0)
    if col_hi < WP:
        nc.vector.memset(xt[parts, rows[0] + 1 : rows[1], col_hi:WP], 0.0)


def build():
    from concourse.tile_rust import add_dep_helper

    nc = bacc.Bacc(
        "TRN2", target_bir_lowering=False, debug=False, enable_partition_id=False
    )
    x_d = nc.dram_tensor("x", [CIN, PIX], F32, kind="ExternalInput")
    w_d = nc.dram_tensor("w9", [CIN, 9, COUT], I32, kind="ExternalInput")
    b_d = nc.dram_tensor("b", [3 * P, 1], I32, kind="ExternalInput")
    out_d = nc.dram_tensor("out", [COUT, PIX], F32, kind="ExternalOutput")

    with tile.TileContext(nc) as tc:
        with (
            tc.tile_pool(name="persist", bufs=1) as persist,
            tc.tile_pool(name="stage", bufs=1) as stage,
            tc.tile_pool(name="outsb", bufs=4) as outsb,
            tc.tile_pool(name="psum", bufs=1, space="PSUM") as psum_pool,
        ):
            # All input DMAs ride the sync queue, which processes them in
            # issue order: earliest-deadline first. hf8 decode runs on DVE,
            # pad casts on the Scalar engine, PSUM epilogue on Scalar.
            # ---- SBUF tiles ----
            wraw = [
                stage.tile([P, 9, COUT], I32, tag=f"wraw{c}", name=f"wraw{c}")
                for c in range(2)
            ]
            wt = [
                persist.tile([P, 9, COUT], F16, tag=f"wl{c}", name=f"wl{c}")
                for c in range(2)
            ]
            xs = [
                stage.tile([P, H, W], F32, tag=f"xstage{c}", name=f"xstage{c}")
                for c in range(2)
            ]
            xt = [
                persist.tile([P, HP, WP], F16, tag=f"xpad{c}", name=f"xpad{c}")
                for c in range(2)
            ]
            wraw2 = stage.tile([P, 5, COUT], I32, tag="wraw2", name="wraw2")
            wpair = persist.tile([P, 5, COUT], F16, tag="wpair", name="wpair")
            xs2 = stage.tile([P, H, W], F32, tag="xstage2", name="xstage2")
            xp2 = persist.tile([P, HP, WP], F16, tag="xpad2", name="xpad2")
            xb2 = persist.tile([P, HP, WP], F16, tag="xpadb", name="xpadb")
            wl = wt
            xp = xt

            # ---- input DMAs, deadline order, one in-order queue ----
            wfl = [w.rearrange("p a b -> p (a b)") for w in wraw]
            half = 5 * COUT
            nc.sync.dma_start(wfl[0][:, :half], w_d[0:P, :5])
            nc.sync.dma_start(
                xs[0][:, : H // 2],
                x_d[0:P, : PIX // 2].rearrange("p (h w) -> p h w", h=H // 2),
            )
            nc.sync.dma_start(wfl[0][:, half:], w_d[0:P, 5:])
            nc.sync.dma_start(
                xs[0][:, H // 2 :],
                x_d[0:P, PIX // 2 :].rearrange("p (h w) -> p h w", h=H // 2),
            )
            nc.sync.dma_start(wraw[1][:], w_d[P : 2 * P])
            nc.sync.dma_start(
                xs[1][:], x_d[P : 2 * P].rearrange("p (h w) -> p h w", h=H)
            )
            cs, ce = 256, 320
            nc.sync.dma_start(wraw2[0:64, 0:5], w_d[cs:ce, 0:9:2])
            nc.sync.dma_start(wraw2[64:128, 0:4], w_d[cs:ce, 1:9:2])
            nc.sync.dma_start(
                xs2[0:64], x_d[cs:ce].rearrange("p (h w) -> p h w", h=H)
            )
            nc.sync.dma_start(
                xs2[64:128], x_d[cs:ce].rearrange("p (h w) -> p h w", h=H)
            )
            braw = stage.tile([P, 3], I32, tag="braw", name="braw")
            nc.sync.dma_start(
                braw[:], b_d.rearrange("(a p) one -> p (a one)", p=P)
            )

            # ---- PE warmup: keep TensorE busy (HAM at 8/8) through the
            # prologue so the real stream starts at 2.4 GHz ----
            wsrc = stage.tile([P, P], F16, tag="wsrc", name="wsrc")
            nc.vector.memset(wsrc[:], 0.0)
            warm_ps = psum_pool.tile([P, N_TILE], F32, tag="acc0", name="warm_ps")
            for _ in range(150):
                nc.tensor.matmul(
                    warm_ps[:, 0:P], wsrc[:], wsrc[:], start=True, stop=True
                )

            # ---- borders (DVE, no data deps: fills the DMA wait) ----
            for c in range(2):
                _pad_borders(nc, xt[c], 2, W + 2)
            _pad_borders(nc, xp2, 2, W + 2, parts=slice(0, 64))
            _pad_borders(nc, xp2, 1, W + 1, parts=slice(64, P))
            _pad_borders(nc, xb2, 2, W + 2, parts=slice(0, 64))
            nc.vector.memset(xb2[64:128, H : HP, :], 0.0)
            nc.vector.memset(xb2[64:128, 0:H, 0:4], 0.0)
            nc.vector.memset(wraw2[64:128, 4], 0)

            # ---- Scalar-engine casts (warm the Copy table first), chained
            # in deadline order so the static schedule matches reality ----
            warm = stage.tile([P, 1], F32, tag="warm", name="warm")
            nc.vector.memset(warm[:], 0.0)
            a0 = nc.scalar.copy(warm[:], warm[:])
            a1 = nc.scalar.copy(
                xt[0][:, 1 : H // 2 + 1, 2 : W + 2], xs[0][:, : H // 2]
            )
            a2 = nc.scalar.copy(
                xt[0][:, H // 2 + 1 : H + 1, 2 : W + 2], xs[0][:, H // 2 :]
            )
            a3 = nc.scalar.copy(xt[1][:, 1 : H + 1, 2 : W + 2], xs[1][:])
            a4 = nc.scalar.copy(xp2[0:64, 1 : H + 1, 2 : W + 2], xs2[0:64])
            a5 = nc.scalar.copy(xp2[64:128, 1 : H + 1, 1 : W + 1], xs2[64:128])
            a6 = nc.scalar.copy(xb2[64:128, 0:H, 4:WP], xs2[64:128])
            prev = a0
            for a in (a1, a2, a3, a4, a5, a6):
                add_dep_helper(a.ins, prev.ins, sync=False, reason="cast order")
                prev = a

            # ---- hf8 decode on DVE, stage-chained in deadline order ----
            d1 = _decode_hf8(
                nc, stage, wfl[0][:, :half],
                wt[0].rearrange("p a b -> p (a b)")[:, :half], P, half, "wdec",
            )
            d2 = _decode_hf8(
                nc, stage, wfl[0][:, half:],
                wt[0].rearrange("p a b -> p (a b)")[:, half:],
                P, 9 * COUT - half, "wdec", after=d1,
            )
            d3 = _decode_hf8(
                nc, stage, wfl[1],
                wt[1].rearrange("p a b -> p (a b)"), P, 9 * COUT, "wdec", after=d2,
            )
            d4 = _decode_hf8(
                nc, stage,
                wraw2.rearrange("p a b -> p (a b)"),
                wpair.rearrange("p a b -> p (a b)"), P, 5 * COUT, "wdec2", after=d3,
            )
            # xb2 lower = same padded image as xp2 lower (same partitions)
            cpy = nc.vector.tensor_copy(
                xb2[0:64, 1 : H + 1, 2 : W + 2], xp2[0:64, 1 : H + 1, 2 : W + 2]
            )
            add_dep_helper(cpy.ins, d4.ins, sync=False, reason="tail copy order")
            bias = []
            prev = None
            for mi, (ms, me) in enumerate(CO_CHUNKS):
                pm = me - ms
                bf = persist.tile([P, 1], F32, tag=f"bias{mi}", name=f"bias{mi}")
                prev = _decode_hf8(
                    nc, stage, braw[:pm, mi : mi + 1], bf[:pm], pm, 1, "bdec",
                    after=prev if prev is not None else d4,
                )
                bias.append(bf)

            # ---- matmuls: out[co, pix] += w[ci,co].T @ x_shift[ci, pix] ----
            n_acc = 2 * 9 + len(TAIL_PAIRS) + 1  # 23 per psum tile
            for mi, (ms, me) in enumerate(CO_CHUNKS):
                pm = me - ms
                acc = [
                    psum_pool.tile(
                        [P, N_TILE], F32, tag=f"acc{t}", name=f"acc_{mi}_{t}"
                    )
                    for t in range(N_PIX_TILES)
                ]
                acc_k = [0] * N_PIX_TILES

                def mm(lhsT, src, kh, kw, t, pm=pm, acc=acc, acc_k=acc_k):
                    h0 = t * ROWS_PER_TILE
                    rhs = src[
                        : lhsT.shape[0],
                        h0 + kh : h0 + kh + ROWS_PER_TILE,
                        kw + 1 : kw + 1 + W,
                    ]
                    nc.tensor.matmul(
                        acc[t][:pm], lhsT, rhs,
                        start=(acc_k[t] == 0), stop=(acc_k[t] == n_acc - 1),
                    )
                    acc_k[t] += 1

                # For the very first co chunk, order chunk-0 work as
                # (weight half x image half) passes: the first 20 matmuls
                # need only the first 5 decoded positions and the first half
                # of the chunk-0 image.
                def tail_mms(t_range, pm=pm):
                    for j, (pa, pb) in enumerate(TAIL_PAIRS):
                        kh, kw = pa // 3, pa % 3
                        src = xb2 if (pa, pb) == (2, 3) else xp2
                        for t in t_range:
                            mm(wpair[:, j, ms:me], src, kh, kw, t)
                    for t in t_range:
                        mm(wpair[0:64, 4, ms:me], xp2, 2, 2, t)

                def epilogue(t, pm=pm, ms=ms, mi=mi):
                    osb = outsb.tile([P, N_TILE], F32, tag="osb", name="osb")
                    nc.scalar.activation(
                        osb[:pm], acc[t][:pm],
                        mybir.ActivationFunctionType.Identity,
                        bias=bias[mi][:pm], scale=1.0,
                    )
                    nc.sync.dma_start(
                        out_d[ms : ms + pm, t * N_TILE : (t + 1) * N_TILE], osb[:pm]
                    )

                if mi < 2:
                    if mi == 0:
                        c0_passes = [
                            (range(0, 5), range(0, 4)),
                            (range(5, 9), range(0, 4)),
                            (range(0, 5), range(4, 8)),
                            (range(5, 9), range(4, 8)),
                        ]
                    else:
                        c0_passes = [(range(9), range(N_PIX_TILES))]
                    for ci in range(2):
                        passes = (
                            c0_passes if ci == 0 else [(range(9), range(N_PIX_TILES))]
                        )
                        for pos_range, t_range in passes:
                            for pos in pos_range:
                                lhsT = wl[ci][:, pos, ms:me]
                                for t in t_range:
                                    mm(lhsT, xp[ci], pos // 3, pos % 3, t)
                    tail_mms(range(N_PIX_TILES))
                    assert all(k == n_acc for k in acc_k)
                    for t in range(N_PIX_TILES):
                        epilogue(t)
                else:
                    # last co chunk tile-by-tile: each PSUM tile finishes its
                    # 23 accumulations early so the Identity+bias epilogue
                    # overlaps the remaining stream instead of trailing it
                    for t in range(N_PIX_TILES):
                        for ci in range(2):
                            for pos in range(9):
                                mm(wl[ci][:, pos, ms:me], xp[ci], pos // 3, pos % 3, t)
                        tail_mms([t])
                        epilogue(t)
                    assert all(k == n_acc for k in acc_k)

    nc.compile()
    return nc


_NC_CACHE = None


def _get_nc():
    global _NC_CACHE
    if _NC_CACHE is None:
        _NC_CACHE = build()
    return _NC_CACHE


def _prep_in_maps(x, w_bits, b_bits):
    # w_bits [co, ci, kh, kw] -> [ci, kh*3+kw, co] (host relayout only)
    w9 = np.ascontiguousarray(
        w_bits.astype(np.int32).transpose(1, 2, 3, 0).reshape(CIN, 9, COUT)
    )
    b2 = np.zeros((3 * 128, 1), np.int32)
    b2[:COUT, 0] = b_bits.astype(np.int32).reshape(COUT)
    return [
        {
            "x": np.ascontiguousarray(x[i].reshape(CIN, PIX).astype(np.float32)),
            "w9": w9,
            "b": b2,
        }
        for i in range(B)
    ]


def kernel(x, w_bits, b_bits):
    nc = _get_nc()
    in_maps = _prep_in_maps(x, w_bits, b_bits)
    res = run_bass_kernel_spmd(nc, in_maps, core_ids=list(range(B)), trace=False)
    return np.stack(
        [res.results[i]["out"].reshape(COUT, H, W) for i in range(B)]
    ).astype(np.float32)


if __name__ == "__main__":
    rng = np.random.default_rng(0)
    x = rng.standard_normal((B, CIN, H, W)).astype(np.float32)
    w_bits = rng.integers(0, 256, (COUT, CIN, 3, 3)).astype(np.int32)
    b_bits = rng.integers(0, 256, (COUT,)).astype(np.int32)
    out = kernel(x, w_bits, b_bits)
    print("out", out.shape, out.dtype, float(np.abs(out).mean()))
